# revision 7
# baseline (speedup 1.0000x reference)
"""Trainium2 Bass kernel for nn_Decision_Node (Linear+Hardtanh -> sp, 2-class
softmax Gini -> gini), data-parallel over 8 NeuronCores.

Math per core shard (B_s=128 of B=1024 batches, T=128, F=784, L=256, C=2):
    sp   = clip(x @ W.T + b, -1, 1)                      [N=16384, 256]
    p0   = sigmoid(sp * d),  d = contrib[...,0]-contrib[...,1]
    gini = 2 - p0^2 - p1^2 = 1 + 2 p0 (1-p0) = 1.5 - 0.5*tanh(sp*d/2)^2

Device strategy:
  - x cast to fp16 on host, column-blocked+padded to [7, N, 128] with a
    bias-fold column (x_pad[6,:,16] = 1.0 pairs with wt[6,16,:] = b).
  - fp16 xT tiles loaded with the xbar DMA-transpose (f on partitions),
    fp16 matmuls with fp32 PSUM accumulation (abs err ~1.5e-3).
  - DVE: clip (one fused max/min tensor_scalar) + z = sp*d.
  - ACT: tanh(z/2), square, affine -> gini.
  - 1 MiB batched stores of sp/gini via staging tiles.
"""

import os
import sys
import types
from concurrent.futures import ThreadPoolExecutor

import numpy as np

for _p in (
    "/opt/trn_rl_repo",
    "/root/.axon_site",
    "/root/.axon_site/_ro/trn_rl_repo",
    "/root/.axon_site/_ro/pypackages",
):
    if os.path.isdir(_p) and _p not in sys.path:
        sys.path.append(_p)

B, T, F, L = 1024, 128, 784, 256
NCORES = 8
BS = B // NCORES          # batches per core
NROWS = BS * T            # 16384 rows per core
KT = 7                    # contraction tiles (784 = 6*128 + 16, padded)


def _build_module(nrows, nb, grp):
    """Build + compile the single-core Bass/Tile module (SPMD across cores)."""
    import concourse.tile as tile
    from concourse import bacc, mybir

    f32, f16 = mybir.dt.float32, mybir.dt.float16
    Alu = mybir.AluOpType
    Act = mybir.ActivationFunctionType

    nc = bacc.Bacc(
        "TRN2",
        target_bir_lowering=False,
        debug=False,
        enable_asserts=False,
        num_devices=NCORES,
    )
    xt_d = nc.dram_tensor("xt", [KT, nrows, 128], f16, kind="ExternalInput").ap()
    wt_d = nc.dram_tensor("wt", [KT, 128, L], f16, kind="ExternalInput").ap()
    d_d = nc.dram_tensor("d8", [T, grp * L], f32, kind="ExternalInput").ap()
    sp_d = nc.dram_tensor("sp", [nrows, L], f32, kind="ExternalOutput").ap()
    gi_d = nc.dram_tensor("gini", [nrows, L], f32, kind="ExternalOutput").ap()

    nblocks = nrows // nb
    tpb = nb // 128       # 128-row tiles per block
    gpb = tpb // grp      # stage groups per block
    GF = grp * L          # free size of one stage group (2048)

    with tile.TileContext(nc) as tc:
        with (
            tc.tile_pool(name="consts", bufs=1) as consts,
            tc.tile_pool(name="xt", bufs=2) as xt_pool,
            tc.tile_pool(name="psum", bufs=8, space="PSUM") as psum_pool,
            tc.tile_pool(name="stage", bufs=2) as stage_pool,
            tc.tile_pool(name="tmp", bufs=2) as tmp_pool,
        ):
            wt_sb = consts.tile([128, KT, L], f16)
            nc.sync.dma_start(wt_sb[:], wt_d.rearrange("k p l -> p k l"))
            d8_sb = consts.tile([128, GF], f32)
            nc.sync.dma_start(d8_sb[:], d_d[:])
            bias15 = consts.tile([128, 1], f32)
            nc.vector.memset(bias15[:], 1.5)

            for blk in range(nblocks):
                n0 = blk * nb
                xts = []
                for k in range(KT):
                    xk = xt_pool.tile([128, nb], f16, tag=f"x{k}")
                    nc.sync.dma_start(
                        xk[:], xt_d[k, n0 : n0 + nb, :], transpose=True
                    )
                    xts.append(xk)
                for g in range(gpb):
                    sp_st = stage_pool.tile([128, grp, L], f32, tag="sp_st")
                    gi_st = stage_pool.tile([128, grp, L], f32, tag="gi_st")
                    z_big = tmp_pool.tile([128, GF], f32, tag="z")
                    for h in range(grp):
                        t = g * grp + h
                        ps = psum_pool.tile([128, L], f32)
                        for k in range(KT):
                            nc.tensor.matmul(
                                ps[:],
                                xts[k][:, t * 128 : (t + 1) * 128],
                                wt_sb[:, k, :],
                                start=(k == 0),
                                stop=(k == KT - 1),
                            )
                        # fused hardtanh: (ps max -1) min 1, PSUM -> stage
                        nc.vector.tensor_scalar(
                            sp_st[:, h, :],
                            ps[:],
                            -1.0,
                            1.0,
                            Alu.max,
                            Alu.min,
                        )
                    sp_flat = sp_st[:].rearrange("p a l -> p (a l)")
                    nc.vector.tensor_tensor(
                        z_big[:], sp_flat, d8_sb[:], Alu.mult
                    )
                    th_big = tmp_pool.tile([128, GF], f32, tag="th")
                    nc.scalar.activation(th_big[:], z_big[:], Act.Tanh, scale=0.5)
                    u_big = tmp_pool.tile([128, GF], f32, tag="u")
                    nc.scalar.activation(u_big[:], th_big[:], Act.Square)
                    nc.scalar.activation(
                        gi_st[:].rearrange("p a l -> p (a l)"),
                        u_big[:],
                        Act.Identity,
                        bias=bias15[:],
                        scale=-0.5,
                    )
                    g0 = n0 + g * grp * 128
                    nc.sync.dma_start(
                        sp_d[g0 : g0 + grp * 128, :].rearrange(
                            "(a p) l -> p a l", p=128
                        ),
                        sp_st[:],
                    )
                    nc.sync.dma_start(
                        gi_d[g0 : g0 + grp * 128, :].rearrange(
                            "(a p) l -> p a l", p=128
                        ),
                        gi_st[:],
                    )

    nc.compile()
    return nc


def _prep_core_x(x_flat_core):
    """[16384, 784] fp32 -> blocked padded fp16 [7, 16384, 128]."""
    xt = np.zeros((KT, x_flat_core.shape[0], 128), np.float16)
    for k in range(6):
        xt[k] = x_flat_core[:, k * 128 : (k + 1) * 128]
    xt[6, :, :16] = x_flat_core[:, 768:784]
    xt[6, :, 16] = 1.0
    return xt


def _prep_wt(W, b):
    wt = np.zeros((KT, 128, L), np.float16)
    WT = W.T  # [784, 256]
    for k in range(6):
        wt[k] = WT[k * 128 : (k + 1) * 128]
    wt[6, :16] = WT[768:784]
    wt[6, 16] = b
    return wt


_module_cache = {}


def _get_module(nrows, nb, grp):
    key = (nrows, nb, grp)
    if key not in _module_cache:
        _module_cache[key] = _build_module(nrows, nb, grp)
    return _module_cache[key]


def _install_ntff_hook():
    """Register the axon NTFF profiling hook missing from this image's antenv."""
    try:
        import antenv.axon_hooks  # noqa: F401

        return
    except ImportError:
        pass
    try:
        from trn_agent_boot.trn_boot import _ntff_profile_via_ctypes

        hook = _ntff_profile_via_ctypes("/opt/axon/libaxon_pjrt.so")
    except Exception:
        hook = None
    mod = types.ModuleType("antenv.axon_hooks")
    mod.get_axon_ntff_profile_hook = lambda: hook
    mod.set_axon_ntff_profile_hook = lambda h: None
    sys.modules["antenv.axon_hooks"] = mod


def _run(x, W, b, contribution, trace=False, tmpdir=None):
    from concourse import bass_utils

    nc = _get_module(NROWS, 2048, 8)

    x_flat = np.ascontiguousarray(x, dtype=np.float32).reshape(NCORES, NROWS, F)
    wt = _prep_wt(np.asarray(W, np.float32), np.asarray(b, np.float32))
    c = np.asarray(contribution, np.float32)
    d = np.ascontiguousarray(c[:, :, 0] - c[:, :, 1], dtype=np.float32)
    d8 = np.ascontiguousarray(np.tile(d, (1, 8)))

    with ThreadPoolExecutor(NCORES) as ex:
        xts = list(ex.map(_prep_core_x, [x_flat[i] for i in range(NCORES)]))

    if trace:
        _install_ntff_hook()
    in_maps = [{"xt": xts[i], "wt": wt, "d8": d8} for i in range(NCORES)]
    res = bass_utils.run_bass_kernel_spmd(
        nc, in_maps, core_ids=list(range(NCORES)), trace=trace, tmpdir=tmpdir
    )

    sp = np.concatenate([res.results[i]["sp"] for i in range(NCORES)], axis=0)
    gini = np.concatenate([res.results[i]["gini"] for i in range(NCORES)], axis=0)
    out = (
        sp.reshape(B, T, L).astype(np.float32, copy=False),
        gini.reshape(B, T, L).astype(np.float32, copy=False),
    )
    return (out, res) if trace else (out, None)


def kernel(x, W, b, contribution):
    out, _ = _run(x, W, b, contribution, trace=False)
    return out


# revision 11
# speedup vs baseline: 1.7297x; 1.7297x over previous
"""Trainium2 Bass kernel for nn_Decision_Node (Linear+Hardtanh -> sp, 2-class
softmax Gini -> gini), data-parallel over 8 NeuronCores.

Math per core shard (B_s=128 of B=1024 batches, T=128, F=784, L=256, C=2):
    sp   = clip(x @ W.T + b, -1, 1)                      [N=16384, 256]
    p0   = sigmoid(sp * d),  d = contrib[...,0]-contrib[...,1]
    gini = 2 - p0^2 - p1^2 = 1 + 2 p0 (1-p0) = 1.5 - 0.5*tanh(sp*d/2)^2

Device strategy:
  - x cast to fp16 on host, column-blocked+padded to [7, N, 128] with a
    bias-fold column (x_pad[6,:,16] = 1.0 pairs with wt[6,16,:] = b).
  - fp16 xT tiles loaded with the xbar DMA-transpose (f on partitions),
    fp16 matmuls with fp32 PSUM accumulation (abs err ~1.5e-3).
  - DVE: clip (one fused max/min tensor_scalar) + z = sp*d.
  - ACT: tanh(z/2), square, affine -> gini.
  - 1 MiB batched stores of sp/gini via staging tiles.
"""

import os
import sys
import types
from concurrent.futures import ThreadPoolExecutor

import numpy as np

for _p in (
    "/opt/trn_rl_repo",
    "/root/.axon_site",
    "/root/.axon_site/_ro/trn_rl_repo",
    "/root/.axon_site/_ro/pypackages",
):
    if os.path.isdir(_p) and _p not in sys.path:
        sys.path.append(_p)

B, T, F, L = 1024, 128, 784, 256
NCORES = 8
BS = B // NCORES          # batches per core
NROWS = BS * T            # 16384 rows per core
KT = 7                    # contraction tiles (784 = 6*128 + 16, padded)


def _build_module(nrows, nb, grp):
    """Build + compile the single-core Bass/Tile module (SPMD across cores)."""
    import concourse.tile as tile
    from concourse import bacc, mybir

    f32, f16 = mybir.dt.float32, mybir.dt.float16
    Alu = mybir.AluOpType
    Act = mybir.ActivationFunctionType

    nc = bacc.Bacc(
        "TRN2",
        target_bir_lowering=False,
        debug=False,
        enable_asserts=False,
        num_devices=NCORES,
    )
    KP = 17  # used partitions in the last (remainder+bias) k-tile
    xt_d = nc.dram_tensor("xt", [KT, 128, nrows], f16, kind="ExternalInput").ap()
    wt_d = nc.dram_tensor("wt", [KT, 128, L], f16, kind="ExternalInput").ap()
    d_d = nc.dram_tensor("d8", [T, grp * L], f32, kind="ExternalInput").ap()
    sp_d = nc.dram_tensor("sp", [nrows, L], f32, kind="ExternalOutput").ap()
    gi_d = nc.dram_tensor("gini", [nrows, L], f32, kind="ExternalOutput").ap()

    nblocks = nrows // nb
    tpb = nb // 128       # 128-row tiles per block
    gpb = tpb // grp      # stage groups per block
    GF = grp * L          # free size of one stage group (2048)

    with tile.TileContext(nc) as tc:
        with (
            tc.tile_pool(name="consts", bufs=1) as consts,
            tc.tile_pool(name="xt", bufs=3) as xt_pool,
            tc.tile_pool(name="psum", bufs=8, space="PSUM") as psum_pool,
            tc.tile_pool(name="stage", bufs=2) as stage_pool,
            tc.tile_pool(name="tmp", bufs=2) as tmp_pool,
        ):
            wt_sb = consts.tile([128, KT, L], f16)
            nc.scalar.dma_start(wt_sb[:], wt_d.rearrange("k p l -> p k l"))
            d8_sb = consts.tile([128, GF], f32)
            nc.scalar.dma_start(d8_sb[:], d_d[:])
            bias15 = consts.tile([128, 1], f32)
            nc.vector.memset(bias15[:], 1.5)

            for blk in range(nblocks):
                n0 = blk * nb
                xts = []
                for k in range(KT):
                    kp = KP if k == KT - 1 else 128
                    xk = xt_pool.tile([kp, nb], f16, tag=f"x{k}")
                    nc.sync.dma_start(xk[:], xt_d[k, 0:kp, n0 : n0 + nb])
                    xts.append(xk)
                for g in range(gpb):
                    sp_st = stage_pool.tile([128, grp, L], f32, tag="sp_st")
                    gi_st = stage_pool.tile([128, grp, L], f32, tag="gi_st")
                    z_big = tmp_pool.tile([128, GF], f32, tag="z")
                    for h in range(grp):
                        t = g * grp + h
                        ps = psum_pool.tile([128, L], f32)
                        for k in range(KT):
                            kp = KP if k == KT - 1 else 128
                            nc.tensor.matmul(
                                ps[:],
                                xts[k][:, t * 128 : (t + 1) * 128],
                                wt_sb[0:kp, k, :],
                                start=(k == 0),
                                stop=(k == KT - 1),
                            )
                        # fused hardtanh: (ps max -1) min 1, PSUM -> stage
                        nc.vector.tensor_scalar(
                            sp_st[:, h, :],
                            ps[:],
                            -1.0,
                            1.0,
                            Alu.max,
                            Alu.min,
                        )
                    sp_flat = sp_st[:].rearrange("p a l -> p (a l)")
                    nc.vector.tensor_tensor(
                        z_big[:], sp_flat, d8_sb[:], Alu.mult
                    )
                    th_big = tmp_pool.tile([128, GF], f32, tag="th")
                    nc.scalar.activation(th_big[:], z_big[:], Act.Tanh, scale=0.5)
                    u_big = tmp_pool.tile([128, GF], f32, tag="u")
                    nc.scalar.activation(u_big[:], th_big[:], Act.Square)
                    nc.scalar.activation(
                        gi_st[:].rearrange("p a l -> p (a l)"),
                        u_big[:],
                        Act.Identity,
                        bias=bias15[:],
                        scale=-0.5,
                    )
                    g0 = n0 + g * grp * 128
                    nc.scalar.dma_start(
                        sp_d[g0 : g0 + grp * 128, :].rearrange(
                            "(a p) l -> p a l", p=128
                        ),
                        sp_st[:],
                    )
                    nc.scalar.dma_start(
                        gi_d[g0 : g0 + grp * 128, :].rearrange(
                            "(a p) l -> p a l", p=128
                        ),
                        gi_st[:],
                    )

    nc.compile()
    return nc


def _prep_core_x(x_flat_core):
    """[16384, 784] fp32 -> transposed fp16 [7, 128, 16384] (f on partitions).

    Row 16 of the last k-tile is the all-ones bias-fold row.
    """
    n = x_flat_core.shape[0]
    xsT16 = x_flat_core.T.astype(np.float16)  # [784, n], one strided pass
    xt = np.zeros((KT, 128, n), np.float16)
    xt[:6] = xsT16[:768].reshape(6, 128, n)
    xt[6, :16] = xsT16[768:784]
    xt[6, 16] = 1.0
    return xt


def _prep_wt(W, b):
    wt = np.zeros((KT, 128, L), np.float16)
    WT = W.T  # [784, 256]
    for k in range(6):
        wt[k] = WT[k * 128 : (k + 1) * 128]
    wt[6, :16] = WT[768:784]
    wt[6, 16] = b
    return wt


_module_cache = {}


def _get_module(nrows, nb, grp):
    key = (nrows, nb, grp)
    if key not in _module_cache:
        _module_cache[key] = _build_module(nrows, nb, grp)
    return _module_cache[key]


def _install_ntff_hook():
    """Register the axon NTFF profiling hook missing from this image's antenv."""
    try:
        import antenv.axon_hooks  # noqa: F401

        return
    except ImportError:
        pass
    try:
        from trn_agent_boot.trn_boot import _ntff_profile_via_ctypes

        hook = _ntff_profile_via_ctypes("/opt/axon/libaxon_pjrt.so")
    except Exception:
        hook = None
    mod = types.ModuleType("antenv.axon_hooks")
    mod.get_axon_ntff_profile_hook = lambda: hook
    mod.set_axon_ntff_profile_hook = lambda h: None
    sys.modules["antenv.axon_hooks"] = mod


def _run(x, W, b, contribution, trace=False, tmpdir=None):
    from concourse import bass_utils

    nc = _get_module(NROWS, 2048, 8)

    x_flat = np.ascontiguousarray(x, dtype=np.float32).reshape(NCORES, NROWS, F)
    wt = _prep_wt(np.asarray(W, np.float32), np.asarray(b, np.float32))
    c = np.asarray(contribution, np.float32)
    d = np.ascontiguousarray(c[:, :, 0] - c[:, :, 1], dtype=np.float32)
    d8 = np.ascontiguousarray(np.tile(d, (1, 8)))

    with ThreadPoolExecutor(NCORES) as ex:
        xts = list(ex.map(_prep_core_x, [x_flat[i] for i in range(NCORES)]))

    if trace:
        _install_ntff_hook()
    in_maps = [{"xt": xts[i], "wt": wt, "d8": d8} for i in range(NCORES)]
    res = bass_utils.run_bass_kernel_spmd(
        nc, in_maps, core_ids=list(range(NCORES)), trace=trace, tmpdir=tmpdir
    )

    sp = np.concatenate([res.results[i]["sp"] for i in range(NCORES)], axis=0)
    gini = np.concatenate([res.results[i]["gini"] for i in range(NCORES)], axis=0)
    out = (
        sp.reshape(B, T, L).astype(np.float32, copy=False),
        gini.reshape(B, T, L).astype(np.float32, copy=False),
    )
    return (out, res) if trace else (out, None)


def kernel(x, W, b, contribution):
    out, _ = _run(x, W, b, contribution, trace=False)
    return out


# revision 13
# speedup vs baseline: 1.9030x; 1.1002x over previous
"""Trainium2 Bass kernel for nn_Decision_Node (Linear+Hardtanh -> sp, 2-class
softmax Gini -> gini), data-parallel over 8 NeuronCores.

Math per core shard (B_s=128 of B=1024 batches, T=128, F=784, L=256, C=2):
    sp   = clip(x @ W.T + b, -1, 1)                      [N=16384, 256]
    p0   = sigmoid(sp * d),  d = contrib[...,0]-contrib[...,1]
    gini = 2 - p0^2 - p1^2 = 1 + 2 p0 (1-p0) = 1.5 - 0.5*tanh(sp*d/2)^2

Device strategy:
  - x cast to fp16 on host, column-blocked+padded to [7, N, 128] with a
    bias-fold column (x_pad[6,:,16] = 1.0 pairs with wt[6,16,:] = b).
  - fp16 xT tiles loaded with the xbar DMA-transpose (f on partitions),
    fp16 matmuls with fp32 PSUM accumulation (abs err ~1.5e-3).
  - DVE: clip (one fused max/min tensor_scalar) + z = sp*d.
  - ACT: tanh(z/2), square, affine -> gini.
  - 1 MiB batched stores of sp/gini via staging tiles.
"""

import os
import sys
import types
from concurrent.futures import ThreadPoolExecutor

import numpy as np

for _p in (
    "/opt/trn_rl_repo",
    "/root/.axon_site",
    "/root/.axon_site/_ro/trn_rl_repo",
    "/root/.axon_site/_ro/pypackages",
):
    if os.path.isdir(_p) and _p not in sys.path:
        sys.path.append(_p)

B, T, F, L = 1024, 128, 784, 256
NCORES = 8
BS = B // NCORES          # batches per core
NROWS = BS * T            # 16384 rows per core
KT = 7                    # contraction tiles (784 = 6*128 + 16, padded)


def _build_module(nrows, nb, grp):
    """Build + compile the single-core Bass/Tile module (SPMD across cores)."""
    import concourse.tile as tile
    from concourse import bacc, mybir

    f32, f16 = mybir.dt.float32, mybir.dt.float16
    Alu = mybir.AluOpType
    Act = mybir.ActivationFunctionType

    nc = bacc.Bacc(
        "TRN2",
        target_bir_lowering=False,
        debug=False,
        enable_asserts=False,
        num_devices=NCORES,
    )
    KP = 17  # used partitions in the last (remainder+bias) k-tile
    xt_d = nc.dram_tensor("xt", [KT, 128, nrows], f16, kind="ExternalInput").ap()
    wt_d = nc.dram_tensor("wt", [KT, 128, L], f16, kind="ExternalInput").ap()
    d_d = nc.dram_tensor("d8", [T, grp * L], f32, kind="ExternalInput").ap()
    sp_d = nc.dram_tensor("sp", [nrows, L], f32, kind="ExternalOutput").ap()
    gi_d = nc.dram_tensor("gini", [nrows, L], f32, kind="ExternalOutput").ap()

    nblocks = nrows // nb
    tpb = nb // 128       # 128-row tiles per block
    gpb = tpb // grp      # stage groups per block
    GF = grp * L          # free size of one stage group (2048)

    with tile.TileContext(nc) as tc:
        with (
            tc.tile_pool(name="consts", bufs=1) as consts,
            tc.tile_pool(name="xt", bufs=3) as xt_pool,
            tc.tile_pool(name="psum", bufs=8, space="PSUM") as psum_pool,
            tc.tile_pool(name="stage", bufs=2) as stage_pool,
            tc.tile_pool(name="tmp", bufs=2) as tmp_pool,
        ):
            wt_sb = consts.tile([128, KT, L], f16)
            nc.scalar.dma_start(wt_sb[:], wt_d.rearrange("k p l -> p k l"))
            d8_sb = consts.tile([128, GF], f32)
            nc.scalar.dma_start(d8_sb[:], d_d[:])

            for blk in range(nblocks):
                n0 = blk * nb
                xts = []
                for k in range(KT):
                    kp = KP if k == KT - 1 else 128
                    xk = xt_pool.tile([kp, nb], f16, tag=f"x{k}")
                    nc.sync.dma_start(xk[:], xt_d[k, 0:kp, n0 : n0 + nb])
                    xts.append(xk)
                for g in range(gpb):
                    sp_st = stage_pool.tile([128, grp, L], f32, tag="sp_st")
                    gi_st = stage_pool.tile([128, grp, L], f32, tag="gi_st")
                    z_big = tmp_pool.tile([128, GF], f32, tag="z")
                    for h in range(grp):
                        t = g * grp + h
                        ps = psum_pool.tile([128, L], f32)
                        for k in range(KT):
                            kp = KP if k == KT - 1 else 128
                            nc.tensor.matmul(
                                ps[:],
                                xts[k][:, t * 128 : (t + 1) * 128],
                                wt_sb[0:kp, k, :],
                                start=(k == 0),
                                stop=(k == KT - 1),
                            )
                        # fused hardtanh: (ps max -1) min 1, PSUM -> stage
                        nc.vector.tensor_scalar(
                            sp_st[:, h, :],
                            ps[:],
                            -1.0,
                            1.0,
                            Alu.max,
                            Alu.min,
                        )
                    sp_flat = sp_st[:].rearrange("p a l -> p (a l)")
                    nc.vector.tensor_tensor(
                        z_big[:], sp_flat, d8_sb[:], Alu.mult
                    )
                    th_big = tmp_pool.tile([128, GF], f32, tag="th")
                    nc.scalar.activation(th_big[:], z_big[:], Act.Tanh, scale=0.5)
                    u_big = tmp_pool.tile([128, GF], f32, tag="u")
                    nc.scalar.activation(u_big[:], th_big[:], Act.Square)
                    # gini = 1.5 - 0.5*u  (fp32 SBUF tensor_scalar runs in 2x mode)
                    nc.vector.tensor_scalar(
                        gi_st[:].rearrange("p a l -> p (a l)"),
                        u_big[:],
                        -0.5,
                        1.5,
                        Alu.mult,
                        Alu.add,
                    )
                    g0 = n0 + g * grp * 128
                    nc.gpsimd.dma_start(
                        sp_d[g0 : g0 + grp * 128, :].rearrange(
                            "(a p) l -> p a l", p=128
                        ),
                        sp_st[:],
                    )
                    nc.gpsimd.dma_start(
                        gi_d[g0 : g0 + grp * 128, :].rearrange(
                            "(a p) l -> p a l", p=128
                        ),
                        gi_st[:],
                    )

    nc.compile()
    return nc


def _prep_core_x(x_flat_core):
    """[16384, 784] fp32 -> transposed fp16 [7, 128, 16384] (f on partitions).

    Row 16 of the last k-tile is the all-ones bias-fold row.
    """
    n = x_flat_core.shape[0]
    xsT16 = x_flat_core.T.astype(np.float16)  # [784, n], one strided pass
    xt = np.zeros((KT, 128, n), np.float16)
    xt[:6] = xsT16[:768].reshape(6, 128, n)
    xt[6, :16] = xsT16[768:784]
    xt[6, 16] = 1.0
    return xt


def _prep_wt(W, b):
    wt = np.zeros((KT, 128, L), np.float16)
    WT = W.T  # [784, 256]
    for k in range(6):
        wt[k] = WT[k * 128 : (k + 1) * 128]
    wt[6, :16] = WT[768:784]
    wt[6, 16] = b
    return wt


_module_cache = {}


def _get_module(nrows, nb, grp):
    key = (nrows, nb, grp)
    if key not in _module_cache:
        _module_cache[key] = _build_module(nrows, nb, grp)
    return _module_cache[key]


def _install_ntff_hook():
    """Register the axon NTFF profiling hook missing from this image's antenv."""
    try:
        import antenv.axon_hooks  # noqa: F401

        return
    except ImportError:
        pass
    try:
        from trn_agent_boot.trn_boot import _ntff_profile_via_ctypes

        hook = _ntff_profile_via_ctypes("/opt/axon/libaxon_pjrt.so")
    except Exception:
        hook = None
    mod = types.ModuleType("antenv.axon_hooks")
    mod.get_axon_ntff_profile_hook = lambda: hook
    mod.set_axon_ntff_profile_hook = lambda h: None
    sys.modules["antenv.axon_hooks"] = mod


def _run(x, W, b, contribution, trace=False, tmpdir=None):
    from concourse import bass_utils

    nc = _get_module(NROWS, 2048, 8)

    x_flat = np.ascontiguousarray(x, dtype=np.float32).reshape(NCORES, NROWS, F)
    wt = _prep_wt(np.asarray(W, np.float32), np.asarray(b, np.float32))
    c = np.asarray(contribution, np.float32)
    d = np.ascontiguousarray(c[:, :, 0] - c[:, :, 1], dtype=np.float32)
    d8 = np.ascontiguousarray(np.tile(d, (1, 8)))

    with ThreadPoolExecutor(NCORES) as ex:
        xts = list(ex.map(_prep_core_x, [x_flat[i] for i in range(NCORES)]))

    if trace:
        _install_ntff_hook()
    in_maps = [{"xt": xts[i], "wt": wt, "d8": d8} for i in range(NCORES)]
    res = bass_utils.run_bass_kernel_spmd(
        nc, in_maps, core_ids=list(range(NCORES)), trace=trace, tmpdir=tmpdir
    )

    sp = np.concatenate([res.results[i]["sp"] for i in range(NCORES)], axis=0)
    gini = np.concatenate([res.results[i]["gini"] for i in range(NCORES)], axis=0)
    out = (
        sp.reshape(B, T, L).astype(np.float32, copy=False),
        gini.reshape(B, T, L).astype(np.float32, copy=False),
    )
    return (out, res) if trace else (out, None)


def kernel(x, W, b, contribution):
    out, _ = _run(x, W, b, contribution, trace=False)
    return out


# revision 19
# speedup vs baseline: 2.2737x; 1.1948x over previous
"""Trainium2 Bass kernel for nn_Decision_Node (Linear+Hardtanh -> sp, 2-class
softmax Gini -> gini), data-parallel over 8 NeuronCores.

Math per core shard (B_s=128 of B=1024 batches, T=128, F=784, L=256, C=2):
    sp   = clip(x @ W.T + b, -1, 1)                      [N=16384, 256]
    p0   = sigmoid(sp * d),  d = contrib[...,0]-contrib[...,1]
    gini = 2 - p0^2 - p1^2 = 1 + 2 p0 (1-p0) = 1.5 - 0.5*tanh(sp*d/2)^2

Device strategy:
  - x cast to fp16 on host, column-blocked+padded to [7, N, 128] with a
    bias-fold column (x_pad[6,:,16] = 1.0 pairs with wt[6,16,:] = b).
  - fp16 xT tiles loaded with the xbar DMA-transpose (f on partitions),
    fp16 matmuls with fp32 PSUM accumulation (abs err ~1.5e-3).
  - DVE: clip (one fused max/min tensor_scalar) + z = sp*d.
  - ACT: tanh(z/2), square, affine -> gini.
  - 1 MiB batched stores of sp/gini via staging tiles.
"""

import os
import sys
import types
from concurrent.futures import ThreadPoolExecutor

import numpy as np

for _p in (
    "/opt/trn_rl_repo",
    "/root/.axon_site",
    "/root/.axon_site/_ro/trn_rl_repo",
    "/root/.axon_site/_ro/pypackages",
):
    if os.path.isdir(_p) and _p not in sys.path:
        sys.path.append(_p)

B, T, F, L = 1024, 128, 784, 256
NCORES = 8
BS = B // NCORES          # batches per core
NROWS = BS * T            # 16384 rows per core
KT = 7                    # contraction tiles (784 = 6*128 + 16, padded)


def _build_module(nrows, nb, grp):
    """Build + compile the single-core Bass/Tile module (SPMD across cores)."""
    import concourse.tile as tile
    from concourse import bacc, mybir

    f32, f16 = mybir.dt.float32, mybir.dt.float16
    Alu = mybir.AluOpType
    Act = mybir.ActivationFunctionType

    nc = bacc.Bacc(
        "TRN2",
        target_bir_lowering=False,
        debug=False,
        enable_asserts=False,
        num_devices=NCORES,
    )
    KP = 17  # used partitions in the last (remainder+bias) k-tile
    xt_d = nc.dram_tensor("xt", [KT, 128, nrows], f16, kind="ExternalInput").ap()
    wt_d = nc.dram_tensor("wt", [KT, 128, L], f16, kind="ExternalInput").ap()
    d_d = nc.dram_tensor("d8", [T, grp * L], f16, kind="ExternalInput").ap()
    sp_d = nc.dram_tensor("sp", [nrows, L], f16, kind="ExternalOutput").ap()
    gi_d = nc.dram_tensor("gini", [nrows, L], f16, kind="ExternalOutput").ap()

    nblocks = nrows // nb
    tpb = nb // 128       # 128-row tiles per block
    gpb = tpb // grp      # stage groups per block
    GF = grp * L          # free size of one stage group (2048)

    with tile.TileContext(nc) as tc:
        with (
            tc.tile_pool(name="consts", bufs=1) as consts,
            tc.tile_pool(name="xt", bufs=4) as xt_pool,
            tc.tile_pool(name="psum", bufs=8, space="PSUM") as psum_pool,
            tc.tile_pool(name="stage", bufs=2) as stage_pool,
            tc.tile_pool(name="tmp", bufs=2) as tmp_pool,
        ):
            wt_sb = consts.tile([128, KT, L], f16)
            nc.scalar.dma_start(wt_sb[:], wt_d.rearrange("k p l -> p k l"))
            d8_sb = consts.tile([128, GF], f16)
            nc.scalar.dma_start(d8_sb[:], d_d[:])

            for blk in range(nblocks):
                n0 = blk * nb
                xts = []
                for k in range(KT):
                    kp = KP if k == KT - 1 else 128
                    xk = xt_pool.tile([kp, nb], f16, tag=f"x{k}")
                    nc.sync.dma_start(xk[:], xt_d[k, 0:kp, n0 : n0 + nb])
                    xts.append(xk)
                for g in range(gpb):
                    sp_st = stage_pool.tile([128, grp, L], f16, tag="sp_st")
                    gi_st = stage_pool.tile([128, grp, L], f16, tag="gi_st")
                    z_big = tmp_pool.tile([128, GF], f16, tag="z")
                    for h in range(grp):
                        t = g * grp + h
                        ps = psum_pool.tile([128, L], f32)
                        for k in range(KT):
                            kp = KP if k == KT - 1 else 128
                            nc.tensor.matmul(
                                ps[:],
                                xts[k][:, t * 128 : (t + 1) * 128],
                                wt_sb[0:kp, k, :],
                                start=(k == 0),
                                stop=(k == KT - 1),
                            )
                        # fused hardtanh: (ps max -1) min 1, PSUM -> stage
                        nc.vector.tensor_scalar(
                            sp_st[:, h, :],
                            ps[:],
                            -1.0,
                            1.0,
                            Alu.max,
                            Alu.min,
                        )
                    sp_flat = sp_st[:].rearrange("p a l -> p (a l)")
                    nc.vector.tensor_tensor(
                        z_big[:], sp_flat, d8_sb[:], Alu.mult
                    )
                    th_big = tmp_pool.tile([128, GF], f32, tag="th")
                    nc.scalar.activation(th_big[:], z_big[:], Act.Tanh, scale=0.5)
                    u_big = tmp_pool.tile([128, GF], f32, tag="u")
                    nc.scalar.activation(u_big[:], th_big[:], Act.Square)
                    # gini = 1.5 - 0.5*u  (fp32 SBUF tensor_scalar runs in 2x mode)
                    nc.vector.tensor_scalar(
                        gi_st[:].rearrange("p a l -> p (a l)"),
                        u_big[:],
                        -0.5,
                        1.5,
                        Alu.mult,
                        Alu.add,
                    )
                    g0 = n0 + g * grp * 128
                    nc.gpsimd.dma_start(
                        sp_d[g0 : g0 + grp * 128, :].rearrange(
                            "(a p) l -> p a l", p=128
                        ),
                        sp_st[:],
                    )
                    nc.gpsimd.dma_start(
                        gi_d[g0 : g0 + grp * 128, :].rearrange(
                            "(a p) l -> p a l", p=128
                        ),
                        gi_st[:],
                    )

    nc.compile()
    return nc


def _prep_core_x(x_flat_core):
    """[16384, 784] fp32 -> transposed fp16 [7, 128, 16384] (f on partitions).

    Row 16 of the last k-tile is the all-ones bias-fold row.
    """
    n = x_flat_core.shape[0]
    xsT16 = x_flat_core.T.astype(np.float16)  # [784, n], one strided pass
    xt = np.zeros((KT, 128, n), np.float16)
    xt[:6] = xsT16[:768].reshape(6, 128, n)
    xt[6, :16] = xsT16[768:784]
    xt[6, 16] = 1.0
    return xt


def _prep_wt(W, b):
    wt = np.zeros((KT, 128, L), np.float16)
    WT = W.T  # [784, 256]
    for k in range(6):
        wt[k] = WT[k * 128 : (k + 1) * 128]
    wt[6, :16] = WT[768:784]
    wt[6, 16] = b
    return wt


_module_cache = {}


def _get_module(nrows, nb, grp):
    key = (nrows, nb, grp)
    if key not in _module_cache:
        _module_cache[key] = _build_module(nrows, nb, grp)
    return _module_cache[key]


def _install_ntff_hook():
    """Register the axon NTFF profiling hook missing from this image's antenv."""
    try:
        import antenv.axon_hooks  # noqa: F401

        return
    except ImportError:
        pass
    try:
        from trn_agent_boot.trn_boot import _ntff_profile_via_ctypes

        hook = _ntff_profile_via_ctypes("/opt/axon/libaxon_pjrt.so")
    except Exception:
        hook = None
    mod = types.ModuleType("antenv.axon_hooks")
    mod.get_axon_ntff_profile_hook = lambda: hook
    mod.set_axon_ntff_profile_hook = lambda h: None
    sys.modules["antenv.axon_hooks"] = mod


def _run(x, W, b, contribution, trace=False, tmpdir=None):
    from concourse import bass_utils

    nc = _get_module(NROWS, 2048, 8)

    x_flat = np.ascontiguousarray(x, dtype=np.float32).reshape(NCORES, NROWS, F)
    wt = _prep_wt(np.asarray(W, np.float32), np.asarray(b, np.float32))
    c = np.asarray(contribution, np.float32)
    d = np.ascontiguousarray(c[:, :, 0] - c[:, :, 1], dtype=np.float32)
    d8 = np.ascontiguousarray(np.tile(d, (1, 8)).astype(np.float16))

    with ThreadPoolExecutor(NCORES) as ex:
        xts = list(ex.map(_prep_core_x, [x_flat[i] for i in range(NCORES)]))

    if trace:
        _install_ntff_hook()
    in_maps = [{"xt": xts[i], "wt": wt, "d8": d8} for i in range(NCORES)]
    res = bass_utils.run_bass_kernel_spmd(
        nc, in_maps, core_ids=list(range(NCORES)), trace=trace, tmpdir=tmpdir
    )

    sp = np.concatenate([res.results[i]["sp"] for i in range(NCORES)], axis=0)
    gini = np.concatenate([res.results[i]["gini"] for i in range(NCORES)], axis=0)
    out = (
        sp.reshape(B, T, L).astype(np.float32),
        gini.reshape(B, T, L).astype(np.float32),
    )
    return (out, res) if trace else (out, None)


def kernel(x, W, b, contribution):
    out, _ = _run(x, W, b, contribution, trace=False)
    return out
